# revision 32
# baseline (speedup 1.0000x reference)
"""Trainium2 Bass kernel for DCKModule (involution / dynamic per-pixel conv).

Math (per image, 1 image per core, 8 cores data-parallel over batch):
  x  = relu(W1p @ guide + bias)                  # (64, 9216)
  df = W2 @ x                                    # (784 = 16 g * 49 tap, 9216)
  out[c,r,j] = sum_k df[g(c),k,r,j] * fpad[c, r+di(k), j+dj(k)] + feature

Mapping (fp16 data, fp32 PSUM accumulation), v2 = 128-partition slot tiling:
- Pixel space (96 r x 96 j) is tiled into 24 slots of (32 rows x 12 cols):
  slot s = (window w = s//3, rowgroup b = s%3) covers rows 32b..32b+31 of
  columns 12w..12w+11.  A CHUNK packs 4 consecutive slots on the 128 SBUF
  partitions (slot q at partitions 32q..32q+31), so every DVE/Pool/PE
  instruction runs 128 partitions wide instead of the baseline's 96
  (1.33x more work per billed row).
- Tap row-shift di is materialized host-side (7 shifted copies of the padded
  feature); the col shift dj is a free-dim offset into an 18-wide halo
  window.  The hot loop has NO partition offsets (hw only allows 32-aligned
  operand bases with restrictive span limits).
- Free-dim order in the hot loop is (j, gc, g) with the group index g
  innermost/stride-1 in ALL operands: this lets the per-column df scatter be
  one contiguous (kk,g) run per rowgroup AND keeps every mult operand
  innermost-stride-1 (DVE fp16 2x perf mode).  The x16 group-channel
  broadcast is a stride-0 gc dim on the df operand (free).
- df is produced per image column j on PE ([96 r, 784] PSUM), evacuated to
  fp16 by ACT, then scattered into chunk-slot slabs [128, (j, kk, g)] with
  small contiguous SBUF->SBUF DMAs (DMA may cross partitions; compute
  engines may not).
- Tap mults: DVE (fp16 2x) for 39 taps, Pool for 10.  Tap accumulation:
  PE identity-matmuls into PSUM fp32 (start tap 0, stop tap 48).
- Residual folded into df via a constant-1 x row and a w2a row adding 1.0
  to every group's center tap.  Output fp16; host casts to fp32 and
  unshuffles the slot layout.
"""

import numpy as np

import concourse.bass as bass
import concourse.mybir as mybir
import concourse.tile as tile
from concourse import bacc, bass_utils

B, C, H, W = 8, 256, 96, 96
K7, PAD, G, GC, R = 7, 3, 16, 16, 64
PIX = H * W               # 9216
BN_EPS = 1e-5
JS = 12                   # slot width (output cols per window)
NW = W // JS              # 8 windows
NK = 6                    # chunks (24 slots / 4)
CH = 2                    # channel halves (128 each)
CHW = C // CH             # 128
GH = G // CH              # 8 groups per half
CENTER = PAD * K7 + PAD   # 24
NO = G * K7 * K7          # 784
FW = JS + K7 - 1          # 18: slot cols incl dj halo
FCH = K7 * FW * GC * GH   # 16128: one (k, ch) fpad chunk per partition
NPH = NK * CH             # 12 phases

F32 = mybir.dt.float32
F16 = mybir.dt.float16
TRACE = False

_CACHE = {}

# taps multiplied on Pool (11 of 49): evenly spread, none at the very start
POOL_TAPS = frozenset({4, 8, 13, 17, 22, 26, 30, 34, 38, 43, 47})


def _slot(k, q):
    """Chunk k, quarter q -> (window, rowgroup)."""
    s = 4 * k + q
    return s // 3, s % 3


def _build_nc():
    nc = bacc.Bacc(None, target_bir_lowering=False)
    fpd_d = nc.dram_tensor("fpd", [128, NPH * FCH], F16, kind="ExternalInput")
    gm_d = nc.dram_tensor("gm", [C, PIX], F16, kind="ExternalInput")
    w1_d = nc.dram_tensor("w1t", [C, R], F16, kind="ExternalInput")
    bias_d = nc.dram_tensor("bias", [R, 1], F32, kind="ExternalInput")
    w2_d = nc.dram_tensor("w2ta", [R + 1, NO], F16, kind="ExternalInput")
    i128_d = nc.dram_tensor("i128", [128, 128], F16, kind="ExternalInput")
    ones_d = nc.dram_tensor("ones", [1, PIX], F16, kind="ExternalInput")
    out_d = nc.dram_tensor("out", [128, NPH * CHW * JS], F16,
                           kind="ExternalOutput")

    with tile.TileContext(nc) as tc:
        with tc.tile_pool(name="persist", bufs=1) as persist, \
             tc.tile_pool(name="gmpool", bufs=2) as gmpool, \
             tc.tile_pool(name="slabpool", bufs=3) as slabpool, \
             tc.tile_pool(name="stagpool", bufs=5) as stagpool, \
             tc.tile_pool(name="prodpool", bufs=6) as prodpool, \
             tc.tile_pool(name="pprodpool", bufs=3) as pprodpool, \
             tc.tile_pool(name="outpool", bufs=2) as outpool, \
             tc.tile_pool(name="fpool", bufs=2) as fpool, \
             tc.tile_pool(name="ps", bufs=1, space="PSUM") as ps:

            w1_sb = persist.tile([128, 2 * R], F16, tag="w1", name="w1sb")
            bias_sb = persist.tile([R, 1], F32, tag="bias", name="biassb")
            w2_sb = persist.tile([R + 1, NO], F16, tag="w2", name="w2sb")
            i128_sb = persist.tile([128, 128], F16, tag="i128", name="i128sb")
            x_sb = persist.tile([R + 1, PIX], F16, tag="x", name="xsb")

            nc.sync.dma_start(out=w1_sb[:, 0:R], in_=w1_d[0:128, :])
            nc.sync.dma_start(out=w1_sb[:, R:2 * R], in_=w1_d[128:256, :])
            nc.sync.dma_start(out=bias_sb[:], in_=bias_d[:])
            nc.sync.dma_start(out=w2_sb[:], in_=w2_d[:])
            nc.sync.dma_start(out=i128_sb[:], in_=i128_d[:])
            # constant-1 row of x folds the +feature residual into df
            nc.sync.dma_start(out=x_sb[R:R + 1, :], in_=ones_d[:])

            DI_SZ = FW * GC * GH      # 2304 elems per di slice

            def fetch_fp(k, ch):
                # 7 per-di DMAs: the DMA device is exclusive, so fine
                # granularity lets scatters/gm interleave with this bulk load
                fch = fpool.tile([128, FCH], F16, tag="fch", name="fch")
                base = (k * CH + ch) * FCH
                for di in range(K7):
                    lo = di * DI_SZ
                    nc.sync.dma_start(
                        out=fch[:, lo:lo + DI_SZ],
                        in_=fpd_d[:, base + lo:base + lo + DI_SZ])
                return fch[:].rearrange("p (di jj gc g) -> p di jj gc g",
                                        di=K7, jj=FW, gc=GC)

            # ---- x = relu(W1p @ guide + bias), fp16, pixel-major (j*96+r)
            XCH = 512

            def fetch_gm(s):
                g0 = gmpool.tile([128, XCH], F16, tag="gma", name="gma")
                g1 = gmpool.tile([128, XCH], F16, tag="gmb", name="gmb")
                lo = s * XCH
                nc.sync.dma_start(out=g0[:], in_=gm_d[0:128, lo:lo + XCH])
                nc.sync.dma_start(out=g1[:], in_=gm_d[128:256, lo:lo + XCH])
                return g0, g1

            def x_chunk(s, gtiles):
                px = ps.tile([R, XCH], F32, tag="xps", name="xps")
                for ct in range(2):
                    nc.tensor.matmul(
                        px[:], w1_sb[:, ct * R:(ct + 1) * R], gtiles[ct][:],
                        start=(ct == 0), stop=(ct == 1))
                # relu+bias on DVE (tensor_scalar) so ACT stays Copy-only:
                # Relu<->Copy act-table swaps cost 1.4us each and serialize
                # the df-scatter chain behind ACT's in-order queue
                nc.vector.tensor_scalar(
                    x_sb[:R, s * XCH:(s + 1) * XCH], px[:],
                    bias_sb[:], 0.0,
                    mybir.AluOpType.add, mybir.AluOpType.max)

            def alloc_slab():
                # [128, (j 12, kk 49, g 16)]
                slab = slabpool.tile([128, JS * NO], F16, tag="df",
                                     name="dfslab")
                return slab

            # df production for a QUAD of image columns j0..j0+3 + scatter.
            # Columns adjacent in the slab free dim share one staging tile,
            # so the partition-crossing scatter is 1-2 DMAs per quad (the
            # HWDGE descriptor generator is a serial 625ns/DMA device —
            # per-column scatters would saturate it).
            def _pieces(w):
                k = (3 * w) // 4
                m = w % 4
                if m == 0:
                    return [(k, 0, 3)]
                if m == 1:
                    return [(k, 3, 1), (k + 1, 0, 2)]
                if m == 2:
                    return [(k, 2, 2), (k + 1, 0, 1)]
                return [(k, 1, 3)]

            def df_quad(j0, slabs, evac_eng=0):
                stag = stagpool.tile([H, 4 * NO], F16, tag="stag",
                                     name="stag")
                for i in range(4):
                    j = j0 + i
                    dfp = ps.tile([H, 1024], F32, tag=f"dfps{j % 2}",
                                  name="dfps")
                    xc = x_sb[:, j * H:(j + 1) * H]
                    nc.tensor.matmul(dfp[:, 0:512], xc, w2_sb[:, 0:512],
                                     start=True, stop=True)
                    nc.tensor.matmul(dfp[:, 512:NO], xc, w2_sb[:, 512:NO],
                                     start=True, stop=True)
                    sl = slice(i * NO, (i + 1) * NO)
                    if evac_eng == 1 and i % 2 == 1:
                        nc.vector.tensor_copy(stag[:, sl], dfp[:, :NO])
                    else:
                        nc.scalar.activation(
                            stag[:, sl], dfp[:, :NO],
                            mybir.ActivationFunctionType.Copy)
                w, jl = j0 // JS, j0 % JS
                boff = 0
                for k, q, nq in _pieces(w):
                    if k not in slabs:
                        slabs[k] = alloc_slab()
                    nc.sync.dma_start(
                        out=slabs[k][32 * q:32 * (q + nq),
                                     jl * NO:(jl + 4) * NO],
                        in_=stag[32 * boff:32 * (boff + nq), :])
                    boff += nq

            # ---- prologue -------------------------------------------------
            # DMA order: gm 0-2 land before the big fpd chunk so PE's x
            # matmuls start immediately; fpd(0,0) isn't needed for ~40us
            slabs = {0: alloc_slab(), 1: alloc_slab()}

            gt = {}
            for s in range(3):
                gt[s] = fetch_gm(s)
            fcur = fetch_fp(0, 0)
            for s in range(3):
                x_chunk(s, gt.pop(s))
            for s in range(3, 5):
                gt[s] = fetch_gm(s)
                x_chunk(s, gt.pop(s))
            # chunk 0 needs cols 0..23 (w0 full + w1 rowgroup 0); alternate
            # the evac between ACT and the otherwise-idle DVE to halve the
            # serial stag chain gating phase 0
            for j0 in range(0, 24, 4):
                df_quad(j0, slabs, evac_eng=1)

            # df quad j0..j0+3 needs x chunk ((j0+3)*96+95)//512; keep x
            # just ahead so quads arrive ~1.5 phases before their chunk.
            work = []
            nx = 5
            for j0 in range(24, PIX // H, 4):
                while nx <= ((j0 + 3) * H + H - 1) // XCH:
                    work.append(("gm", nx))
                    work.append(("x", nx))
                    nx += 1
                work.append(("df", j0))
            while nx < PIX // XCH:
                work.append(("gm", nx))
                work.append(("x", nx))
                nx += 1

            def drain_one(slabs):
                if not work:
                    return
                item = work.pop(0)
                if item[0] == "gm":
                    gt[item[1]] = fetch_gm(item[1])
                elif item[0] == "x":
                    x_chunk(item[1], gt.pop(item[1]))
                else:
                    df_quad(item[1], slabs)

            # ---- main loop ------------------------------------------------
            for k in range(NK):
                slab = slabs[k]
                sv = slab[:].rearrange("p (j kk g) -> p j kk g",
                                       j=JS, kk=K7 * K7)
                for ch in range(CH):
                    ph = k * CH + ch
                    last_phase = (ph == NPH - 1)
                    nidx = ph + 1
                    fnxt = (fetch_fp(nidx // CH, nidx % CH)
                            if nidx < NPH else None)
                    acc = ps.tile([128, CHW * JS], F32, tag="acc", name="acc")

                    n_em = [0]

                    def id_adds(prod):
                        # PSUM accumulation is order-independent; only the
                        # first/last EMITTED matmuls carry start/stop
                        first = n_em[0] == 0
                        n_em[0] += 1
                        last = n_em[0] == K7 * K7
                        for s in range(CHW * JS // 512):
                            nc.tensor.matmul(
                                acc[:, s * 512:(s + 1) * 512], i128_sb[:],
                                prod[:, s * 512:(s + 1) * 512],
                                start=first, stop=last)

                    # pool-tap id-adds are deferred one full pool tap so PE
                    # never waits on Pool's slow (3.2us) multiplies
                    dq, pq = [], []
                    for kk in range(K7 * K7):
                        di, dj = divmod(kk, K7)
                        on_pool = kk in POOL_TAPS
                        if last_phase:
                            # few, early pool taps: Pool's 3.2us multiplies
                            # must not stretch the final drain tail
                            on_pool = (kk % 5 == 2 and kk < 30)
                        if on_pool:
                            prod = pprodpool.tile([128, CHW * JS], F16,
                                                  tag="pprod", name="pprod")
                        else:
                            prod = prodpool.tile([128, CHW * JS], F16,
                                                 tag="prod", name="prod")
                        # free order (j, gc, g), g innermost stride 1
                        in0 = fcur[:, di, dj:dj + JS, :, :]
                        in1 = sv[:, :, kk, ch * GH:(ch + 1) * GH] \
                            .unsqueeze(2).broadcast_to((128, JS, GC, GH))
                        pv = prod[:].rearrange("p (j gc g) -> p j gc g",
                                               j=JS, gc=GC)
                        eng = nc.gpsimd if on_pool else nc.vector
                        eng.tensor_tensor(pv, in0, in1,
                                          mybir.AluOpType.mult)
                        (pq if on_pool else dq).append(prod)
                        if len(dq) > 3:
                            id_adds(dq.pop(0))
                        if len(pq) > 1:
                            id_adds(pq.pop(0))
                        if kk % 4 == 1:
                            drain_one(slabs)
                    for p in dq:
                        id_adds(p)
                    for p in pq:
                        id_adds(p)
                    # df work here keeps PE's queue non-empty through the
                    # evac / next-phase-mult-warmup window (p-state guard)
                    drain_one(slabs)
                    drain_one(slabs)
                    ev = outpool.tile([128, CHW * JS], F16, tag="ev",
                                      name="ev")
                    obase = ph * CHW * JS
                    # slice-pipelined evac: the next phase's first id-add
                    # only waits on slice 0's copy, not the whole evac
                    for s in range(CHW * JS // 512):
                        sl = slice(s * 512, (s + 1) * 512)
                        nc.scalar.activation(
                            ev[:, sl], acc[:, sl],
                            mybir.ActivationFunctionType.Copy)
                        nc.sync.dma_start(
                            out=out_d[:, obase + s * 512:
                                      obase + (s + 1) * 512],
                            in_=ev[:, sl])
                    fcur = fnxt
                del slabs[k]
    if not nc.is_finalized():
        nc.finalize()
    return nc


def _host_weights(W1, bn_gamma, bn_beta, bn_mean, bn_var, W2):
    inv = bn_gamma / np.sqrt(bn_var + BN_EPS)
    w1t = np.ascontiguousarray((W1 * inv[:, None]).T).astype(np.float16)
    bias = (bn_beta - bn_mean * inv).astype(np.float32).reshape(R, 1)
    # w2a columns ordered (kk-tap major, group minor): col = kk*16 + g
    w2ta = np.zeros((R + 1, NO), np.float16)
    w2 = W2.reshape(G, K7 * K7, R).transpose(1, 0, 2)  # [kk, g, R]
    w2ta[:R] = w2.reshape(NO, R).T.astype(np.float16)
    w2ta[R, CENTER * G:(CENTER + 1) * G] = 1.0
    i128 = np.eye(128, dtype=np.float16)
    return w1t, bias, w2ta, i128


def _host_fpd(fm4):
    """[b, 128, (phase 12, di 7, jj 18, gc 16, g 8)] slot-tiled fpad."""
    fpad = np.pad(fm4, ((0, 0), (0, 0), (PAD, PAD), (PAD, PAD))) \
        .astype(np.float16)  # [b, 256, 102, 102]
    # channels as [ch 2, g 8, gc 16]
    fpg = fpad.reshape(B, CH, GH, GC, H + 2 * PAD, W + 2 * PAD)
    out = np.empty((B, 128, NPH * FCH), np.float16)
    for k in range(NK):
        for q in range(4):
            w, b = _slot(k, q)
            for ch in range(CH):
                base = (k * CH + ch) * FCH
                for di in range(K7):
                    # rows 32b+di .. +32, cols 12w .. +18
                    blk = fpg[:, ch, :, :, 32 * b + di:32 * b + di + 32,
                              12 * w:12 * w + FW]
                    # [b, g 8, gc 16, 32 r, 18 jj] -> [b, 32, jj, gc, g]
                    blk = blk.transpose(0, 3, 4, 2, 1)
                    lo = base + di * FW * GC * GH
                    out[:, 32 * q:32 * q + 32, lo:lo + FW * GC * GH] = \
                        blk.reshape(B, 32, -1)
    return out


def kernel(feature_map, guide_map, W1, bn_gamma, bn_beta, bn_mean, bn_var, W2):
    fm4 = np.asarray(feature_map, np.float32).reshape(B, C, H, W)
    fpd = _host_fpd(fm4)
    gm = np.ascontiguousarray(
        np.asarray(guide_map, np.float32).reshape(B, C, H, W)
        .transpose(0, 1, 3, 2)).reshape(B, C, PIX).astype(np.float16)
    w1t, bias, w2ta, i128 = _host_weights(
        np.asarray(W1, np.float32), np.asarray(bn_gamma, np.float32),
        np.asarray(bn_beta, np.float32), np.asarray(bn_mean, np.float32),
        np.asarray(bn_var, np.float32), np.asarray(W2, np.float32))

    if "nc" not in _CACHE:
        _CACHE["nc"] = _build_nc()
    nc = _CACHE["nc"]

    ones = np.ones((1, PIX), np.float16)
    in_maps = [dict(fpd=fpd[i], gm=gm[i], w1t=w1t, bias=bias,
                    w2ta=w2ta, i128=i128, ones=ones) for i in range(B)]
    _CACHE["in_maps"] = in_maps
    res = bass_utils.run_bass_kernel_spmd(
        nc, in_maps, core_ids=list(range(B)), trace=TRACE)
    _CACHE["last"] = res
    raw = np.stack([r["out"] for r in res.results], axis=0)
    # [b, 128, (phase, j 12, gc 16, g 8)] -> (b, 256, 96, 96)
    raw = raw.reshape(B, 128, NPH, JS, GC, GH).astype(np.float32)
    out = np.empty((B, C, H, W), np.float32)
    for k in range(NK):
        for q in range(4):
            w, b = _slot(k, q)
            for ch in range(CH):
                ph = k * CH + ch
                blk = raw[:, 32 * q:32 * q + 32, ph]  # [b, 32 r, 12 j, gc, g]
                # channel c = 128*ch + g*16 + gc
                blk = blk.transpose(0, 4, 3, 1, 2)    # [b, g, gc, r, j]
                out[:, ch * CHW:(ch + 1) * CHW,
                    32 * b:32 * b + 32,
                    12 * w:12 * w + JS] = blk.reshape(B, CHW, 32, JS)
    return out
